# revision 23
# baseline (speedup 1.0000x reference)
"""Trainium2 Bass kernel for a single-head cross-attention block.

Reference computation (per batch b of B=128):
    q = input[b] @ Wq            # [T,H]   T=512, C=384, H=64
    k = x[b] @ Wk                # [T,H]
    v = x[b] @ Wv                # [T,H]
    S = (q @ k.T) * C**-0.5      # [T,T], causal mask
    P = softmax(S, axis=-1)
    out[b] = P @ v               # [T,H]

Strategy: data-parallel over 8 NeuronCores (16 batches each).

Host-side layouts are partition-major so every DMA moves one large
contiguous run per partition (6 KB/partition per input batch — input
and x interleaved into a single DMA — and ~1 KB for the output).

Device, per batch (software-pipelined one stage deep so the PE never
idles waiting on softmax):
  - qT|kT = Wq'.T@inpT / Wk'.T@xT -> one 2-bank PSUM [64, 2T]
  - v[t]  = xT[:,tchunk].T @ Wv'  -> PSUM [128,TK,H]
  - Causal S^T chunks (chunk m holds keys 128m..128m+128 vs queries
    128m..512) packed into ONE PSUM tile [128, 1280]: the four 128-wide
    diagonal blocks first (bank 0), then off-diagonal remainders at
    bank-aligned offsets. exp runs as 3 bank-aligned activations.
  - The causal mask costs NO vector/scalar work: before each diagonal
    S^T matmul, a 2-matmul accumulation chain seeds the PSUM block with
    -600 on strictly-lower entries (triM.T @ I), so exp(scale*(S-600))
    ~= 5e-14 kills masked scores inside the existing exp.
  - PV of the PREVIOUS batch is emitted between this batch's qkv and
    S^T so the exp latency hides under ~2us of PE work. A ones-column
    in v_sb makes the softmax denominator fall out of the same matmuls;
    the denominator ships to the host, which divides.
"""

import numpy as np
import ml_dtypes

import concourse.bass as bass
import concourse.tile as tile
import concourse.mybir as mybir
from concourse.bass_utils import run_bass_kernel_spmd
from concourse.masks import make_upper_triangular

N_CORES = 8
B, T, C, H = 128, 512, 384, 64
BPC = B // N_CORES          # batches per core
CK = C // 128               # contraction chunks for projections
TK = T // 128               # T chunks
SCALE = float(C) ** -0.5
BF16 = mybir.dt.bfloat16
F32 = mybir.dt.float32
EXP = mybir.ActivationFunctionType.Exp

_bf16 = ml_dtypes.bfloat16

# Packed layout of causal S^T chunks inside one [128, 1280] PSUM tile.
# Diagonal 128x128 blocks of chunks 0..3 at [128m : 128m+128] = bank 0.
# Off-diagonal remainders (chunk m covers queries 128(m+1)..512):
#   m0 rest (384 f32) at 512, m2 rest (128) at 896  -> bank 1
#   m1 rest (256) at 1024                           -> bank 2
# No matmul output crosses a 2KB PSUM bank boundary.
DIAG_BASE = [0, 128, 256, 384]
REST_BASE = {0: 512, 1: 1024, 2: 896}
ST_PACKED = 1280
EXP_RANGES = [(0, 512), (512, 1024), (1024, 1280)]


def _split_multi_waits(nc: bass.Bass):
    """walrus in this build encodes at most ONE sync-wait per instruction.
    Tile's wait-assignment can attach several. Move the extras onto
    same-engine NOPs inserted immediately before each instruction —
    identical semantics (the engine blocks on the NOP waits first)."""
    n = 0
    for bb in nc.m.functions[0].blocks:
        new_insts = []
        for inst in bb.instructions:
            si = inst.sync_info
            waits = list(si.on_wait) if si and si.on_wait else []
            if len(waits) > 1:
                for w in waits[:-1]:
                    nop = mybir.InstNoOp(name=f"WSPLIT-{n}", ins=[], outs=[])
                    n += 1
                    nop.engine = inst.engine
                    nop.sync_info = mybir.SyncInfo(on_wait=[w], on_update=[])
                    new_insts.append(nop)
                si.on_wait = waits[-1:]
            new_insts.append(inst)
        bb.instructions[:] = new_insts


def build_kernel() -> bass.Bass:
    nc = bass.Bass()
    inx = nc.dram_tensor("inx", [BPC, 128, 2, CK, T], BF16,
                         kind="ExternalInput")
    wall = nc.dram_tensor("wall", [128, 3, CK, H], BF16, kind="ExternalInput")
    out = nc.dram_tensor("out", [BPC, 128, TK, H + 1], F32,
                         kind="ExternalOutput")

    PREFETCH = 2

    with tile.TileContext(nc) as tc:
        with (
            tc.tile_pool(name="const", bufs=1) as const_pool,
            tc.tile_pool(name="inputs", bufs=PREFETCH + 1) as in_pool,
            tc.tile_pool(name="qk", bufs=2) as qk_pool,
            tc.tile_pool(name="e", bufs=2) as e_pool,
            tc.tile_pool(name="osb", bufs=4) as o_pool,
            tc.tile_pool(name="qk_ps", bufs=1, space="PSUM") as qk_psum,
            tc.tile_pool(name="st_ps", bufs=1, space="PSUM") as st_psum,
            tc.tile_pool(name="v_ps", bufs=2, space="PSUM") as v_psum,
            tc.tile_pool(name="o_ps", bufs=1, space="PSUM") as o_psum,
        ):
            w_sb = const_pool.tile([128, 3, CK, H], BF16, tag="wall")
            nc.sync.dma_start(w_sb[:], wall[:])
            # 4x-tiled upper-triangular (incl diagonal) 0/1 causal mask
            tri4 = const_pool.tile([128, 4 * 128], BF16, tag="tri4")
            for j in range(TK):
                make_upper_triangular(nc, tri4[:, 128 * j:128 * (j + 1)],
                                      val=1.0, diag=True)
            # two persistent v tiles (ones column written once each)
            v_tiles = [const_pool.tile([128, TK, H + 1], BF16, tag=f"v{i}",
                                       name=f"v{i}")
                       for i in range(2)]
            for vt in v_tiles:
                nc.gpsimd.memset(vt[:, :, H], 1.0)

            in_tiles = {}
            state = {}

            def emit_load(b):
                it = in_pool.tile([128, 2, CK, T], BF16, tag="inx")
                nc.sync.dma_start(it[:], inx[b])
                in_tiles[b] = it

            def emit_qkv(b):
                inxt = in_tiles.pop(b)
                it = inxt[:, 0]
                xt = inxt[:, 1]
                v_ps = v_psum.tile([128, TK, H], F32, tag="v")
                for t in range(TK):
                    for c in range(CK):
                        nc.tensor.matmul(
                            v_ps[:, t, :],
                            xt[:, c, 128 * t:128 * (t + 1)],
                            w_sb[:, 2, c, :],
                            start=(c == 0), stop=(c == CK - 1),
                        )
                v_sb = v_tiles[b % 2]
                nc.vector.tensor_copy(v_sb[:, :, 0:H], v_ps[:])

                qk_ps = qk_psum.tile([H, 2 * T], F32, tag="qk")
                qk_sb = qk_pool.tile([H, 2 * T], BF16, tag="qk_sb")
                for c in range(CK):
                    nc.tensor.matmul(
                        qk_ps[:, 0:T], w_sb[:, 0, c, :], it[:, c, :],
                        start=(c == 0), stop=(c == CK - 1),
                    )
                for c in range(CK):
                    nc.tensor.matmul(
                        qk_ps[:, T:2 * T], w_sb[:, 1, c, :], xt[:, c, :],
                        start=(c == 0), stop=(c == CK - 1),
                    )
                nc.vector.tensor_copy(qk_sb[:, 0:T], qk_ps[:, 0:T])
                nc.vector.tensor_copy(qk_sb[:, T:2 * T], qk_ps[:, T:2 * T])
                state[b] = [qk_sb, v_sb]

            def emit_st(b):
                qk_sb, _ = state[b]
                qT = qk_sb[:, 0:T]
                kT = qk_sb[:, T:2 * T]
                st_ps = st_psum.tile([128, ST_PACKED], F32, tag="st")
                e = e_pool.tile([128, ST_PACKED], BF16, tag="e")
                e2 = e_pool.tile([128, 512], BF16, tag="e2")
                for m in range(TK):
                    n0 = 128 * m
                    nc.tensor.matmul(
                        st_ps[:, DIAG_BASE[m]:DIAG_BASE[m] + 128],
                        kT[:, n0:n0 + 128], qT[:, n0:n0 + 128],
                        start=True, stop=True,
                    )
                lo, hi = EXP_RANGES[0]
                nc.scalar.activation(e[:, lo:hi], st_ps[:, lo:hi], EXP,
                                     scale=SCALE)
                nc.vector.tensor_mul(e2[:], e[:, 0:512], tri4[:])
                for m, ncols in ((0, 384), (2, 128), (1, 256)):
                    n0 = 128 * m
                    nc.tensor.matmul(
                        st_ps[:, REST_BASE[m]:REST_BASE[m] + ncols],
                        kT[:, n0:n0 + 128], qT[:, 128 * (m + 1):T],
                        start=True, stop=True,
                    )
                for lo, hi in EXP_RANGES[1:]:
                    nc.scalar.activation(e[:, lo:hi], st_ps[:, lo:hi], EXP,
                                         scale=SCALE)
                state[b].append((e, e2))

            def emit_pv(b):
                _, v_sb, (e, e2) = state.pop(b)
                o_ps = o_psum.tile([128, TK, H + 1], F32, tag="o")
                for t in range(TK):
                    for m in range(t + 1):
                        if m == t:
                            src_e = e2[:, DIAG_BASE[m]:DIAG_BASE[m] + 128]
                        else:
                            off = REST_BASE[m] + 128 * (t - m - 1)
                            src_e = e[:, off:off + 128]
                        nc.tensor.matmul(
                            o_ps[:, t, :],
                            src_e,
                            v_sb[:, m, :],
                            start=(m == 0), stop=(m == t),
                        )
                o_sb = o_pool.tile([128, TK, H + 1], F32, tag="o_sb")
                nc.scalar.copy(o_sb[:], o_ps[:])
                nc.sync.dma_start(out[b], o_sb[:])

            for b in range(min(PREFETCH, BPC)):
                emit_load(b)
            for b in range(BPC):
                if b + PREFETCH < BPC:
                    emit_load(b + PREFETCH)
                emit_qkv(b)
                if b > 0:
                    emit_pv(b - 1)
                emit_st(b)
            emit_pv(BPC - 1)
    _split_multi_waits(nc)
    return nc


def _layout_input(a: np.ndarray) -> np.ndarray:
    """[n, T, C] f32 -> [n, 128, CK, T] bf16, partition-major."""
    a = np.asarray(a, dtype=np.float32)
    n = a.shape[0]
    a = a.transpose(0, 2, 1).reshape(n, CK, 128, T).transpose(0, 2, 1, 3)
    return np.ascontiguousarray(a).astype(_bf16)


def _layout_weights(Wq, Wk, Wv) -> np.ndarray:
    """three [C, H] -> [128, 3, CK, H] bf16."""
    def lay(w):
        w = np.asarray(w, dtype=np.float32)
        return w.reshape(CK, 128, H).transpose(1, 0, 2)
    return np.ascontiguousarray(
        np.stack([lay(Wq), lay(Wk), lay(Wv)], axis=1)).astype(_bf16)


def prepare_in_maps(input, x, Wq, Wk, Wv):
    inpT = _layout_input(input)
    xT = _layout_input(x)
    # interleave per (batch, partition): [n, 128, 2, CK, T]
    inx = np.ascontiguousarray(np.stack([inpT, xT], axis=2))
    wall = _layout_weights(Wq, Wk, Wv)
    in_maps = []
    for c in range(N_CORES):
        sl = slice(c * BPC, (c + 1) * BPC)
        in_maps.append({
            "inx": np.ascontiguousarray(inx[sl]),
            "wall": wall,
        })
    return in_maps


def postprocess(results) -> np.ndarray:
    # device layout [BPC, 128, TK, H+1]: [b, p, t, 0:H] is the unnormalized
    # PV sum for query row 128t+p; [..., H] is the softmax denominator.
    outs = []
    for r in results:
        raw = r["out"].reshape(BPC, 128, TK, H + 1)
        o = raw[..., :H] / raw[..., H:]
        outs.append(o.transpose(0, 2, 1, 3).reshape(BPC, T, H))
    return np.concatenate(outs, axis=0).astype(np.float32)


_cached_nc = None


def kernel(input: np.ndarray, x: np.ndarray, Wq: np.ndarray, Wk: np.ndarray,
           Wv: np.ndarray) -> np.ndarray:
    global _cached_nc
    if _cached_nc is None:
        _cached_nc = build_kernel()
    nc = _cached_nc
    in_maps = prepare_in_maps(input, x, Wq, Wk, Wv)
    res = run_bass_kernel_spmd(nc, in_maps, core_ids=list(range(N_CORES)))
    return postprocess(res.results)


# revision 25
# speedup vs baseline: 1.0312x; 1.0312x over previous
"""Trainium2 Bass kernel for a single-head cross-attention block.

Reference computation (per batch b of B=128):
    q = input[b] @ Wq            # [T,H]   T=512, C=384, H=64
    k = x[b] @ Wk                # [T,H]
    v = x[b] @ Wv                # [T,H]
    S = (q @ k.T) * C**-0.5      # [T,T], causal mask
    P = softmax(S, axis=-1)
    out[b] = P @ v               # [T,H]

Strategy: data-parallel over 8 NeuronCores (16 batches each).

Host-side layouts are partition-major so every DMA moves one large
contiguous run per partition (6 KB/partition per input batch — input
and x interleaved into a single DMA — and ~1 KB for the output).

Device, per batch (software-pipelined one stage deep so the PE never
idles waiting on softmax):
  - qT|kT = Wq'.T@inpT / Wk'.T@xT -> one 2-bank PSUM [64, 2T]
  - v[t]  = xT[:,tchunk].T @ Wv'  -> PSUM [128,TK,H]
  - Causal S^T chunks (chunk m holds keys 128m..128m+128 vs queries
    128m..512) packed into ONE PSUM tile [128, 1280]: the four 128-wide
    diagonal blocks first (bank 0), then off-diagonal remainders at
    bank-aligned offsets. exp runs as 3 bank-aligned activations.
  - The causal mask costs NO vector/scalar work: before each diagonal
    S^T matmul, a 2-matmul accumulation chain seeds the PSUM block with
    -600 on strictly-lower entries (triM.T @ I), so exp(scale*(S-600))
    ~= 5e-14 kills masked scores inside the existing exp.
  - PV of the PREVIOUS batch is emitted between this batch's qkv and
    S^T so the exp latency hides under ~2us of PE work. A ones-column
    in v_sb makes the softmax denominator fall out of the same matmuls;
    the denominator ships to the host, which divides.
"""

import numpy as np
import ml_dtypes

import concourse.bass as bass
import concourse.tile as tile
import concourse.mybir as mybir
from concourse.bass_utils import run_bass_kernel_spmd
from concourse.masks import make_upper_triangular

N_CORES = 8
B, T, C, H = 128, 512, 384, 64
BPC = B // N_CORES          # batches per core
CK = C // 128               # contraction chunks for projections
TK = T // 128               # T chunks
SCALE = float(C) ** -0.5
BF16 = mybir.dt.bfloat16
F32 = mybir.dt.float32
EXP = mybir.ActivationFunctionType.Exp

_bf16 = ml_dtypes.bfloat16

# Packed layout of causal S^T chunks inside one [128, 1280] PSUM tile.
# Diagonal 128x128 blocks of chunks 0..3 at [128m : 128m+128] = bank 0.
# Off-diagonal remainders (chunk m covers queries 128(m+1)..512):
#   m0 rest (384 f32) at 512, m2 rest (128) at 896  -> bank 1
#   m1 rest (256) at 1024                           -> bank 2
# No matmul output crosses a 2KB PSUM bank boundary.
DIAG_BASE = [0, 128, 256, 384]
REST_BASE = {0: 512, 1: 1024, 2: 896}
ST_PACKED = 1280
EXP_RANGES = [(0, 512), (512, 1024), (1024, 1280)]


def _split_multi_waits(nc: bass.Bass):
    """walrus in this build encodes at most ONE sync-wait per instruction.
    Tile's wait-assignment can attach several. Move the extras onto
    same-engine NOPs inserted immediately before each instruction —
    identical semantics (the engine blocks on the NOP waits first)."""
    n = 0
    for bb in nc.m.functions[0].blocks:
        new_insts = []
        for inst in bb.instructions:
            si = inst.sync_info
            waits = list(si.on_wait) if si and si.on_wait else []
            if len(waits) > 1:
                for w in waits[:-1]:
                    nop = mybir.InstNoOp(name=f"WSPLIT-{n}", ins=[], outs=[])
                    n += 1
                    nop.engine = inst.engine
                    nop.sync_info = mybir.SyncInfo(on_wait=[w], on_update=[])
                    new_insts.append(nop)
                si.on_wait = waits[-1:]
            new_insts.append(inst)
        bb.instructions[:] = new_insts


def build_kernel() -> bass.Bass:
    nc = bass.Bass()
    inx = nc.dram_tensor("inx", [BPC, 128, 2, CK, T], BF16,
                         kind="ExternalInput")
    wall = nc.dram_tensor("wall", [128, 3, CK, H], BF16, kind="ExternalInput")
    out = nc.dram_tensor("out", [BPC, 128, TK, H + 1], F32,
                         kind="ExternalOutput")

    PREFETCH = 2

    with tile.TileContext(nc) as tc:
        with (
            tc.tile_pool(name="const", bufs=1) as const_pool,
            tc.tile_pool(name="inputs", bufs=PREFETCH + 1) as in_pool,
            tc.tile_pool(name="qk", bufs=2) as qk_pool,
            tc.tile_pool(name="e", bufs=2) as e_pool,
            tc.tile_pool(name="osb", bufs=4) as o_pool,
            tc.tile_pool(name="qk_ps", bufs=1, space="PSUM") as qk_psum,
            tc.tile_pool(name="st_ps", bufs=1, space="PSUM") as st_psum,
            tc.tile_pool(name="v_ps", bufs=1, space="PSUM") as v_psum,
            tc.tile_pool(name="o_ps", bufs=2, space="PSUM") as o_psum,
        ):
            w_sb = const_pool.tile([128, 3, CK, H], BF16, tag="wall")
            nc.sync.dma_start(w_sb[:], wall[:])
            # 4x-tiled upper-triangular (incl diagonal) 0/1 causal mask
            tri4 = const_pool.tile([128, 4 * 128], BF16, tag="tri4")
            for j in range(TK):
                make_upper_triangular(nc, tri4[:, 128 * j:128 * (j + 1)],
                                      val=1.0, diag=True)
            # two persistent v tiles (ones column written once each)
            v_tiles = [const_pool.tile([128, TK, H + 1], BF16, tag=f"v{i}",
                                       name=f"v{i}")
                       for i in range(2)]
            for vt in v_tiles:
                nc.gpsimd.memset(vt[:, :, H], 1.0)

            in_tiles = {}
            state = {}

            def emit_load(b):
                it = in_pool.tile([128, 2, CK, T], BF16, tag="inx")
                nc.sync.dma_start(it[:], inx[b])
                in_tiles[b] = it

            def emit_qkv(b):
                inxt = in_tiles.pop(b)
                it = inxt[:, 0]
                xt = inxt[:, 1]
                qk_ps = qk_psum.tile([H, 2 * T], F32, tag="qk")
                qk_sb = qk_pool.tile([H, 2 * T], BF16, tag="qk_sb")
                for c in range(CK):
                    nc.tensor.matmul(
                        qk_ps[:, 0:T], w_sb[:, 0, c, :], it[:, c, :],
                        start=(c == 0), stop=(c == CK - 1),
                    )
                for c in range(CK):
                    nc.tensor.matmul(
                        qk_ps[:, T:2 * T], w_sb[:, 1, c, :], xt[:, c, :],
                        start=(c == 0), stop=(c == CK - 1),
                    )
                nc.vector.tensor_copy(qk_sb[:, 0:T], qk_ps[:, 0:T])
                nc.vector.tensor_copy(qk_sb[:, T:2 * T], qk_ps[:, T:2 * T])

                v_ps = v_psum.tile([128, TK, H], F32, tag="v")
                for t in range(TK):
                    for c in range(CK):
                        nc.tensor.matmul(
                            v_ps[:, t, :],
                            xt[:, c, 128 * t:128 * (t + 1)],
                            w_sb[:, 2, c, :],
                            start=(c == 0), stop=(c == CK - 1),
                        )
                v_sb = v_tiles[b % 2]
                nc.vector.tensor_copy(v_sb[:, :, 0:H], v_ps[:])
                state[b] = [qk_sb, v_sb]

            def emit_st(b):
                qk_sb, _ = state[b]
                qT = qk_sb[:, 0:T]
                kT = qk_sb[:, T:2 * T]
                st_ps = st_psum.tile([128, ST_PACKED], F32, tag="st")
                e = e_pool.tile([128, ST_PACKED], BF16, tag="e")
                e2 = e_pool.tile([128, 512], BF16, tag="e2")
                for m in range(TK):
                    n0 = 128 * m
                    nc.tensor.matmul(
                        st_ps[:, DIAG_BASE[m]:DIAG_BASE[m] + 128],
                        kT[:, n0:n0 + 128], qT[:, n0:n0 + 128],
                        start=True, stop=True,
                    )
                lo, hi = EXP_RANGES[0]
                nc.scalar.activation(e[:, lo:hi], st_ps[:, lo:hi], EXP,
                                     scale=SCALE)
                nc.vector.tensor_mul(e2[:], e[:, 0:512], tri4[:])
                for m, ncols in ((0, 384), (2, 128), (1, 256)):
                    n0 = 128 * m
                    nc.tensor.matmul(
                        st_ps[:, REST_BASE[m]:REST_BASE[m] + ncols],
                        kT[:, n0:n0 + 128], qT[:, 128 * (m + 1):T],
                        start=True, stop=True,
                    )
                for lo, hi in EXP_RANGES[1:]:
                    nc.scalar.activation(e[:, lo:hi], st_ps[:, lo:hi], EXP,
                                         scale=SCALE)
                state[b].append((e, e2))

            def emit_pv(b, split_store=False):
                _, v_sb, (e, e2) = state.pop(b)
                o_ps = o_psum.tile([128, TK, H + 1], F32, tag="o")
                o_sb = o_pool.tile([128, TK, H + 1], F32, tag="o_sb")
                for t in range(TK):
                    for m in range(t + 1):
                        if m == t:
                            src_e = e2[:, DIAG_BASE[m]:DIAG_BASE[m] + 128]
                        else:
                            off = REST_BASE[m] + 128 * (t - m - 1)
                            src_e = e[:, off:off + 128]
                        nc.tensor.matmul(
                            o_ps[:, t, :],
                            src_e,
                            v_sb[:, m, :],
                            start=(m == 0), stop=(m == t),
                        )
                    if split_store:
                        # last batch: drain each query chunk as soon as
                        # its PV chain closes, overlapping the remaining
                        # chains instead of serializing after them.
                        nc.scalar.copy(o_sb[:, t, :], o_ps[:, t, :])
                        nc.sync.dma_start(out[b, :, t], o_sb[:, t, :])
                if not split_store:
                    nc.scalar.copy(o_sb[:], o_ps[:])
                    nc.sync.dma_start(out[b], o_sb[:])

            for b in range(min(PREFETCH, BPC)):
                emit_load(b)
            for b in range(BPC):
                if b + PREFETCH < BPC:
                    emit_load(b + PREFETCH)
                emit_qkv(b)
                if b > 0:
                    emit_pv(b - 1)
                emit_st(b)
            emit_pv(BPC - 1, split_store=True)
    _split_multi_waits(nc)
    return nc


def _layout_input(a: np.ndarray) -> np.ndarray:
    """[n, T, C] f32 -> [n, 128, CK, T] bf16, partition-major."""
    a = np.asarray(a, dtype=np.float32)
    n = a.shape[0]
    a = a.transpose(0, 2, 1).reshape(n, CK, 128, T).transpose(0, 2, 1, 3)
    return np.ascontiguousarray(a).astype(_bf16)


def _layout_weights(Wq, Wk, Wv) -> np.ndarray:
    """three [C, H] -> [128, 3, CK, H] bf16."""
    def lay(w):
        w = np.asarray(w, dtype=np.float32)
        return w.reshape(CK, 128, H).transpose(1, 0, 2)
    return np.ascontiguousarray(
        np.stack([lay(Wq), lay(Wk), lay(Wv)], axis=1)).astype(_bf16)


def prepare_in_maps(input, x, Wq, Wk, Wv):
    inpT = _layout_input(input)
    xT = _layout_input(x)
    # interleave per (batch, partition): [n, 128, 2, CK, T]
    inx = np.ascontiguousarray(np.stack([inpT, xT], axis=2))
    wall = _layout_weights(Wq, Wk, Wv)
    in_maps = []
    for c in range(N_CORES):
        sl = slice(c * BPC, (c + 1) * BPC)
        in_maps.append({
            "inx": np.ascontiguousarray(inx[sl]),
            "wall": wall,
        })
    return in_maps


def postprocess(results) -> np.ndarray:
    # device layout [BPC, 128, TK, H+1]: [b, p, t, 0:H] is the unnormalized
    # PV sum for query row 128t+p; [..., H] is the softmax denominator.
    outs = []
    for r in results:
        raw = r["out"].reshape(BPC, 128, TK, H + 1)
        o = raw[..., :H] / raw[..., H:]
        outs.append(o.transpose(0, 2, 1, 3).reshape(BPC, T, H))
    return np.concatenate(outs, axis=0).astype(np.float32)


_cached_nc = None


def kernel(input: np.ndarray, x: np.ndarray, Wq: np.ndarray, Wk: np.ndarray,
           Wv: np.ndarray) -> np.ndarray:
    global _cached_nc
    if _cached_nc is None:
        _cached_nc = build_kernel()
    nc = _cached_nc
    in_maps = prepare_in_maps(input, x, Wq, Wk, Wv)
    res = run_bass_kernel_spmd(nc, in_maps, core_ids=list(range(N_CORES)))
    return postprocess(res.results)
